# revision 10
# baseline (speedup 1.0000x reference)
"""Trainium2 Bass kernel for NEGATRegressorJAX (k-hop GNN + simplicial edge branch).

Strategy (8-core SPMD, row-sharded, sparse):
  The dense adjacency A [4096^2] and Hodge Laplacians Ll/Lu [8192^2] have only
  8192 / 32768 nonzeros, so every L @ z product is computed sparsely per core
  on its destination-row shard:
    - source rows z[v_e] for this core's edges are brought into SBUF as
      "gather slots" (128-edge chunks across the 128 partitions),
    - per 128-row destination block, selector matmuls on TensorE compute
      z_next^T[f, dest] += Zg_chunk^T[f, e] @ S_chunk[e, dest], where
      S[e, dest] = w_e one-hot on the edge's destination row.  This applies
      the edge weight and segment-sums duplicate destinations in fp32 PSUM.
  Layer-0 hop-1 slots are pure input permutations, so the host pre-gathers
  them and the device loads them with one contiguous DMA.  All other hops
  dma_gather from AllGathered z tables (4 SWDGE queues in parallel; the Q7
  descriptor-emission rate is the limit, ~8ns/descriptor on one queue).
  z @ W_k terms run as W^T @ z^T matmuls (512-wide), bias+ReLU on ScalarE,
  heads on TensorE.  Host work is index/layout preparation only (dedup to
  match JAX scatter set semantics, edge->chunk assignment, int16 index
  tiles, input permutation).
"""

import ml_dtypes
import numpy as np

import concourse.bass as bass
import concourse.bacc as bacc
import concourse.tile as tile
import concourse.mybir as mybir
from concourse import bass_utils
from concourse.masks import make_identity

F32 = mybir.dt.float32
I16 = mybir.dt.int16
RELU = mybir.ActivationFunctionType.Relu

BF16 = True   # hop-path dtype: bf16 tables/selectors (4x PE, half AG bytes)
DT = mybir.dt.bfloat16 if BF16 else mybir.dt.float32
DT_NP = np.dtype(ml_dtypes.bfloat16) if BF16 else np.dtype(np.float32)

N_NODES = 4096
N_EDGES = 8192
NCORES = 8
NS = N_NODES // NCORES    # 512 node rows per core
ES = N_EDGES // NCORES    # 1024 edge rows per core
NBLK_N = NS // 128        # 4
NBLK_E = ES // 128        # 8
P = 128
TW = 128                  # uniform gather-table width (cols)
NQ = 4                    # SWDGE queues


# ----------------------------------------------------------------------------
# host-side index preprocessing
# ----------------------------------------------------------------------------

def _dedup_last(u, v, w, ncols):
    """Match jnp .at[u,v].set(w) semantics: duplicate (u,v) -> last write wins."""
    u = u.astype(np.int64)
    v = v.astype(np.int64)
    key = u * ncols + v
    _, first_rev = np.unique(key[::-1], return_index=True)
    keep = np.sort(len(key) - 1 - first_rev)
    return u[keep], v[keep], w[keep]


def _shard_edges(u, v, w, rows_per_core, nblk):
    """Split edges by destination row shard; order by destination block."""
    cores = []
    C = 1
    for c in range(NCORES):
        m = (u >= c * rows_per_core) & (u < (c + 1) * rows_per_core)
        uu = u[m] - c * rows_per_core
        vv = v[m]
        ww = w[m]
        blk = uu // P
        o = np.argsort(blk, kind="stable")
        uu, vv, ww, blk = uu[o], vv[o], ww[o], blk[o]
        counts = np.bincount(blk, minlength=nblk)
        if counts.max() > 0:
            C = max(C, -(-int(counts.max()) // P))
        cores.append((uu, vv, ww, blk, counts))
    return cores, C


def _chunk_tables(core_edges, nblk, C, vmap):
    """Flat gather index list + selector matrix for one core.

    Edge j of chunk g sits at gather slot g*128+j (partition j of chunk g)
    and contributes S[j, g*128 + u_local%128] = w."""
    uu, vv, ww, blk, counts = core_edges
    G = nblk * C
    idx_flat = np.zeros(G * P, np.int64)
    S = np.zeros((P, G * P), np.float32)
    if len(uu):
        starts = np.zeros(nblk, np.int64)
        starts[1:] = np.cumsum(counts)[:-1]
        pos = np.arange(len(uu)) - starts[blk]
        g = blk * C + pos // P
        part = pos % P
        idx_flat[g * P + part] = vmap(vv)
        S[part, g * P + (uu % P)] = ww
    return idx_flat, S


def _idx_tile(idx_flat):
    """int16 gather-index tile [128, n/16]: element i at [i%16, i//16]."""
    a = idx_flat.astype(np.int16).reshape(-1, 16).T
    return np.ascontiguousarray(np.tile(a, (8, 1)))


def _combined_map(v):
    """Row of zl[v] in the AllGathered (zl || zu) per-core-concatenated table."""
    return 2 * ES * (v // ES) + (v % ES)


def _pregather(table_np, idx_flat, f):
    """Host-side slot materialisation for layer-0 hop-1: [128, G, f]."""
    G = len(idx_flat) // P
    z = table_np[idx_flat].reshape(G, P, f).transpose(1, 0, 2)
    return np.ascontiguousarray(z.astype(DT_NP))


# ----------------------------------------------------------------------------
# device program
# ----------------------------------------------------------------------------

def _build(nc, GA, CA, GL, CL, GU, CU):
    def inp(name, shape, dtype=F32):
        return nc.dram_tensor(name, list(shape), dtype, kind="ExternalInput")

    x_rows = inp("x_rows", [NS, 64])
    ex_rows = inp("ex_rows", [ES, 32])

    # pre-gathered layer-0 hop-1 slots
    zgA0 = inp("zgA0", [P, GA, 64], DT)
    zgL0 = inp("zgL0", [P, GL, 32], DT)
    zgU0 = inp("zgU0", [P, GU, 32], DT)

    idxA = inp("idxA", [P, GA * 8], I16)
    S_A = inp("S_A", [P, GA * P], DT)
    idxL1 = inp("idxL1", [P, GL * 8], I16)
    idxL2 = inp("idxL2", [P, GL * 8], I16)
    S_L = inp("S_L", [P, GL * P], DT)
    idxU1 = inp("idxU1", [P, GU * 8], I16)
    idxU2 = inp("idxU2", [P, GU * 8], I16)
    S_U = inp("S_U", [P, GU * P], DT)

    nW0 = inp("nW0", [3, 64, 128])
    nW1 = inp("nW1", [3, 128, 128])
    eWl0 = inp("eWl0", [3, 32, 128])
    eWu0 = inp("eWu0", [3, 32, 128])
    eWl1 = inp("eWl1", [3, 128, 128])
    eWu1 = inp("eWu1", [3, 128, 128])
    fnW = inp("fnW", [128, 64])
    feW = inp("feW", [128, 64])
    bn0 = inp("bn0", [128, 1])
    bn1 = inp("bn1", [128, 1])
    be0 = inp("be0", [128, 1])
    be1 = inp("be1", [128, 1])
    fnb_rep = inp("fnb_rep", [128, 64])
    feb_rep = inp("feb_rep", [128, 64])

    node_out = nc.dram_tensor("node_out", [NS, 64], F32, kind="ExternalOutput")
    edge_out = nc.dram_tensor("edge_out", [ES, 64], F32, kind="ExternalOutput")

    # AllGather bounce buffers (Local) and gathered tables (Shared), all [*, TW]
    def bounce(name, rows):
        return nc.dram_tensor(name, [rows, TW], DT, kind="Internal")

    def table(name, rows):
        return nc.dram_tensor(name, [rows, TW], DT, kind="Internal",
                              addr_space="Shared")

    bn_z1_0 = bounce("bn_z1_0", NS)
    bn_h1 = bounce("bn_h1", NS)
    bn_z1_1 = bounce("bn_z1_1", NS)
    be_z1_0 = bounce("be_z1_0", 2 * ES)
    be_h1 = bounce("be_h1", ES)
    be_z1_1 = bounce("be_z1_1", 2 * ES)
    T_nz1_0 = table("T_nz1_0", N_NODES)
    T_nh1 = table("T_nh1", N_NODES)
    T_nz1_1 = table("T_nz1_1", N_NODES)
    T_ez1_0 = table("T_ez1_0", 2 * N_EDGES)
    T_eh1 = table("T_eh1", N_EDGES)
    T_ez1_1 = table("T_ez1_1", 2 * N_EDGES)

    RG = [list(range(NCORES))]
    qrr = [0]

    with tile.TileContext(nc) as tc:
        with tc.tile_pool(name="const", bufs=1) as cp, \
             tc.tile_pool(name="work", bufs=3) as wp, \
             tc.tile_pool(name="psum", bufs=2, space="PSUM") as pp:

            ident = cp.tile([P, P], F32, tag="ident")
            make_identity(nc, ident[:])
            zpad = cp.tile([P, 96], DT, tag="zpad")
            nc.vector.memset(zpad[:], 0.0)

            def load_const(ap, shape, tag, dtype=F32):
                t = cp.tile(list(shape), dtype, tag=tag, name=tag)
                nc.sync.dma_start(out=t[:], in_=ap)
                return t

            sA = load_const(S_A[:], [P, GA * P], "sA", DT)
            sL = load_const(S_L[:], [P, GL * P], "sL", DT)
            sU = load_const(S_U[:], [P, GU * P], "sU", DT)
            iA = load_const(idxA[:], [P, GA * 8], "iA", I16)
            iL1 = load_const(idxL1[:], [P, GL * 8], "iL1", I16)
            iL2 = load_const(idxL2[:], [P, GL * 8], "iL2", I16)
            iU1 = load_const(idxU1[:], [P, GU * 8], "iU1", I16)
            iU2 = load_const(idxU2[:], [P, GU * 8], "iU2", I16)

            w_n0 = [load_const(nW0[k], [64, 128], f"wn0_{k}") for k in range(3)]
            w_n1 = [load_const(nW1[k], [128, 128], f"wn1_{k}") for k in range(3)]
            w_el0 = [load_const(eWl0[k], [32, 128], f"wel0_{k}") for k in range(3)]
            w_eu0 = [load_const(eWu0[k], [32, 128], f"weu0_{k}") for k in range(3)]
            w_el1 = [load_const(eWl1[k], [128, 128], f"wel1_{k}") for k in range(3)]
            w_eu1 = [load_const(eWu1[k], [128, 128], f"weu1_{k}") for k in range(3)]
            w_fn = load_const(fnW[:], [128, 64], "wfn")
            w_fe = load_const(feW[:], [128, 64], "wfe")
            b_n0 = load_const(bn0[:], [128, 1], "bn0")
            b_n1 = load_const(bn1[:], [128, 1], "bn1")
            b_e0 = load_const(be0[:], [128, 1], "be0")
            b_e1 = load_const(be1[:], [128, 1], "be1")
            b_fn = load_const(fnb_rep[:], [128, 64], "bfn")
            b_fe = load_const(feb_rep[:], [128, 64], "bfe")

            def hop(src, idx_t, s_t, G, C, nblk, f_in, zT, zg_tag):
                """zT[f, u] = sum_e S[e, u] * z[idx_e, f] on this core's rows.

                src: (table, elem) to dma_gather from, or an input AP holding
                pre-gathered slots [P, G, f_in]."""
                if isinstance(src, tuple):
                    tbl, elem = src
                    zg = wp.tile([P, G, elem], DT, tag=zg_tag)
                    for p0 in range(0, G, 8):
                        gcnt = min(8, G - p0)
                        nidx = gcnt * P
                        nc.gpsimd.dma_gather(
                            zg[:, p0:p0 + gcnt, :], tbl[:],
                            idx_t[:, p0 * 8:(p0 + gcnt) * 8], nidx, nidx, elem,
                            queue_num=qrr[0] % NQ)
                        qrr[0] += 1
                else:
                    zg = wp.tile([P, G, f_in], DT, tag=zg_tag)
                    nc.sync.dma_start(out=zg[:], in_=src)
                for b in range(nblk):
                    ps = pp.tile([f_in, P], F32, tag="ps_hop")
                    for cc in range(C):
                        g = b * C + cc
                        nc.tensor.matmul(
                            ps[:],
                            lhsT=zg[:, g, :f_in],
                            rhs=s_t[:, g * P:(g + 1) * P],
                            start=(cc == 0),
                            stop=(cc == C - 1),
                        )
                    nc.vector.tensor_copy(out=zT[:, b * P:(b + 1) * P], in_=ps[:])

            def transpose_in(rows_ap, R, f_in, zT):
                """Load [R, f_in] DRAM rows and produce zT [f_in, R] in SBUF."""
                for t in range(R // P):
                    rt = wp.tile([P, f_in], F32, tag="rowin")
                    nc.sync.dma_start(out=rt[:], in_=rows_ap[t * P:(t + 1) * P, :])
                    ps = pp.tile([f_in, P], F32, tag="ps_t")
                    nc.tensor.transpose(ps[:], rt[:], ident[:])
                    nc.vector.tensor_copy(out=zT[:, t * P:(t + 1) * P], in_=ps[:])

            def rowify(zT, f, R, bnc, row_off=0):
                """zT [f, R] -> row-major [R, TW] (DT, zero-padded) into bounce."""
                for t in range(R // P):
                    ps = pp.tile([P, f], F32, tag="ps_t")
                    nc.tensor.transpose(ps[:], zT[:, t * P:(t + 1) * P],
                                        ident[:f, :f])
                    rt = wp.tile([P, TW], DT, tag="rowout")
                    nc.vector.tensor_copy(out=rt[:, :f], in_=ps[:])
                    if TW > f:
                        nc.vector.tensor_copy(out=rt[:, f:], in_=zpad[:, :TW - f])
                    nc.sync.dma_start(
                        out=bnc[row_off + t * P: row_off + (t + 1) * P, :],
                        in_=rt[:])

            def allgather(bnc, table_t):
                nc.gpsimd.collective_compute(
                    "AllGather", mybir.AluOpType.bypass, replica_groups=RG,
                    ins=[bnc[:]], outs=[table_t[:]])

            def acc_relu(terms, bias, R, hT):
                """hT[128, R] = relu(sum_k W_k^T @ zT_k + bias)."""
                for t in range(R // 512):
                    ps = pp.tile([P, 512], F32, tag="ps_acc")
                    for i, (zT, w, f_in) in enumerate(terms):
                        nc.tensor.matmul(
                            ps[:], lhsT=w[:], rhs=zT[:, t * 512:(t + 1) * 512],
                            start=(i == 0), stop=(i == len(terms) - 1))
                    nc.scalar.activation(hT[:, t * 512:(t + 1) * 512], ps[:],
                                         RELU, bias=bias[:])

            def head(hT, w, b_rep, R, out_t):
                for t in range(R // P):
                    ps = pp.tile([P, 64], F32, tag="ps_t")
                    nc.tensor.matmul(ps[:], lhsT=hT[:, t * P:(t + 1) * P],
                                     rhs=w[:], start=True, stop=True)
                    rt = wp.tile([P, 64], F32, tag="rowout_f32")
                    nc.vector.tensor_add(out=rt[:], in0=ps[:], in1=b_rep[:])
                    nc.sync.dma_start(out=out_t[t * P:(t + 1) * P, :], in_=rt[:])

            def persist(shape, tag):
                return cp.tile(list(shape), F32, tag=tag, name=tag)

            # ---------------- layer 0 (hop-1 slots pre-gathered) -------------
            x0T = persist([64, NS], "x0T")
            transpose_in(x_rows[:], NS, 64, x0T)
            e0T = persist([32, ES], "e0T")
            transpose_in(ex_rows[:], ES, 32, e0T)

            ezl1 = persist([32, ES], "ezl1")
            ezu1 = persist([32, ES], "ezu1")
            nz1 = persist([64, NS], "nz1")
            hop(zgL0[:], iL1, sL, GL, CL, NBLK_E, 32, ezl1, "zg_e")
            hop(zgU0[:], iU1, sU, GU, CU, NBLK_E, 32, ezu1, "zg_e")
            hop(zgA0[:], iA, sA, GA, CA, NBLK_N, 64, nz1, "zg_n")
            rowify(ezl1, 32, ES, be_z1_0, row_off=0)
            rowify(ezu1, 32, ES, be_z1_0, row_off=ES)
            allgather(be_z1_0, T_ez1_0)
            rowify(nz1, 64, NS, bn_z1_0)
            allgather(bn_z1_0, T_nz1_0)

            ezl2 = persist([32, ES], "ezl2")
            ezu2 = persist([32, ES], "ezu2")
            nz2 = persist([64, NS], "nz2")
            hop((T_ez1_0, TW), iL2, sL, GL, CL, NBLK_E, 32, ezl2, "zg_e")
            hop((T_ez1_0, TW), iU2, sU, GU, CU, NBLK_E, 32, ezu2, "zg_e")
            hop((T_nz1_0, TW), iA, sA, GA, CA, NBLK_N, 64, nz2, "zg_n")

            eh1 = persist([128, ES], "eh1")
            acc_relu([(e0T, w_el0[0], 32), (ezl1, w_el0[1], 32), (ezl2, w_el0[2], 32),
                      (e0T, w_eu0[0], 32), (ezu1, w_eu0[1], 32), (ezu2, w_eu0[2], 32)],
                     b_e0, ES, eh1)
            rowify(eh1, 128, ES, be_h1)
            allgather(be_h1, T_eh1)

            nh1 = persist([128, NS], "nh1")
            acc_relu([(x0T, w_n0[0], 64), (nz1, w_n0[1], 64), (nz2, w_n0[2], 64)],
                     b_n0, NS, nh1)
            rowify(nh1, 128, NS, bn_h1)
            allgather(bn_h1, T_nh1)

            # ---------------- layer 1 ----------------------------------------
            ezl1b = persist([128, ES], "ezl1b")
            ezu1b = persist([128, ES], "ezu1b")
            nz1b = persist([128, NS], "nz1b")
            hop((T_eh1, TW), iL1, sL, GL, CL, NBLK_E, 128, ezl1b, "zg_e")
            hop((T_eh1, TW), iU1, sU, GU, CU, NBLK_E, 128, ezu1b, "zg_e")
            hop((T_nh1, TW), iA, sA, GA, CA, NBLK_N, 128, nz1b, "zg_n")
            rowify(ezl1b, 128, ES, be_z1_1, row_off=0)
            rowify(ezu1b, 128, ES, be_z1_1, row_off=ES)
            allgather(be_z1_1, T_ez1_1)
            rowify(nz1b, 128, NS, bn_z1_1)
            allgather(bn_z1_1, T_nz1_1)

            ezl2b = persist([128, ES], "ezl2b")
            ezu2b = persist([128, ES], "ezu2b")
            nz2b = persist([128, NS], "nz2b")
            hop((T_ez1_1, TW), iL2, sL, GL, CL, NBLK_E, 128, ezl2b, "zg_e")
            hop((T_ez1_1, TW), iU2, sU, GU, CU, NBLK_E, 128, ezu2b, "zg_e")
            hop((T_nz1_1, TW), iA, sA, GA, CA, NBLK_N, 128, nz2b, "zg_n")

            eh2 = persist([128, ES], "eh2")
            acc_relu([(eh1, w_el1[0], 128), (ezl1b, w_el1[1], 128),
                      (ezl2b, w_el1[2], 128), (eh1, w_eu1[0], 128),
                      (ezu1b, w_eu1[1], 128), (ezu2b, w_eu1[2], 128)],
                     b_e1, ES, eh2)
            nh2 = persist([128, NS], "nh2")
            acc_relu([(nh1, w_n1[0], 128), (nz1b, w_n1[1], 128),
                      (nz2b, w_n1[2], 128)], b_n1, NS, nh2)
            head(eh2, w_fe, b_fe, ES, edge_out)
            head(nh2, w_fn, b_fn, NS, node_out)


# ----------------------------------------------------------------------------
# entry point
# ----------------------------------------------------------------------------

def prepare(x, edge_x, node_edge_index, edge_index_l, edge_attr_l,
            edge_index_u, edge_attr_u,
            nW0, nb0, nW1, nb1, fnW, fnb,
            eWl0, eWu0, eb0, eWl1, eWu1, eb1, feW, feb):
    """Build + compile the SPMD program and per-core input maps."""
    x = np.asarray(x, np.float32)
    edge_x = np.asarray(edge_x, np.float32)
    nei = np.asarray(node_edge_index)
    eil = np.asarray(edge_index_l)
    eiu = np.asarray(edge_index_u)
    eal = np.asarray(edge_attr_l, np.float32)
    eau = np.asarray(edge_attr_u, np.float32)

    uA, vA, wA = _dedup_last(nei[0], nei[1], np.ones(nei.shape[1], np.float32),
                             N_NODES)
    uL, vL, wL = _dedup_last(eil[0], eil[1], eal, N_EDGES)
    uU, vU, wU = _dedup_last(eiu[0], eiu[1], eau, N_EDGES)

    coresA, CA = _shard_edges(uA, vA, wA, NS, NBLK_N)
    coresL, CL = _shard_edges(uL, vL, wL, ES, NBLK_E)
    coresU, CU = _shard_edges(uU, vU, wU, ES, NBLK_E)
    GA, GL, GU = NBLK_N * CA, NBLK_E * CL, NBLK_E * CU

    nc = bacc.Bacc("TRN2", target_bir_lowering=False, debug=False,
                   num_devices=NCORES, num_swdge_queues=NQ)
    _build(nc, GA, CA, GL, CL, GU, CU)
    nc.compile()

    shared = {
        "nW0": np.asarray(nW0, np.float32),
        "nW1": np.asarray(nW1, np.float32),
        "eWl0": np.asarray(eWl0, np.float32),
        "eWu0": np.asarray(eWu0, np.float32),
        "eWl1": np.asarray(eWl1, np.float32),
        "eWu1": np.asarray(eWu1, np.float32),
        "fnW": np.asarray(fnW, np.float32),
        "feW": np.asarray(feW, np.float32),
        "bn0": np.asarray(nb0, np.float32).sum(0).reshape(128, 1),
        "bn1": np.asarray(nb1, np.float32).sum(0).reshape(128, 1),
        "be0": np.asarray(eb0, np.float32).reshape(128, 1),
        "be1": np.asarray(eb1, np.float32).reshape(128, 1),
        "fnb_rep": np.ascontiguousarray(
            np.broadcast_to(np.asarray(fnb, np.float32), (128, 64))),
        "feb_rep": np.ascontiguousarray(
            np.broadcast_to(np.asarray(feb, np.float32), (128, 64))),
    }

    ident = lambda v: v
    in_maps = []
    for c in range(NCORES):
        iA_f, S_A = _chunk_tables(coresA[c], NBLK_N, CA, ident)
        iL1_f, S_L = _chunk_tables(coresL[c], NBLK_E, CL, ident)
        iL2_f, _ = _chunk_tables(coresL[c], NBLK_E, CL, _combined_map)
        iU1_f, S_U = _chunk_tables(coresU[c], NBLK_E, CU, ident)
        iU2_f, _ = _chunk_tables(coresU[c], NBLK_E, CU,
                                 lambda v: _combined_map(v) + ES)
        in_maps.append({
            **shared,
            "x_rows": x[c * NS:(c + 1) * NS, :].copy(),
            "ex_rows": edge_x[c * ES:(c + 1) * ES, :].copy(),
            "zgA0": _pregather(x, iA_f, 64),
            "zgL0": _pregather(edge_x, iL1_f, 32),
            "zgU0": _pregather(edge_x, iU1_f, 32),
            "idxA": _idx_tile(iA_f),
            "S_A": S_A.astype(DT_NP),
            "idxL1": _idx_tile(iL1_f), "idxL2": _idx_tile(iL2_f),
            "S_L": S_L.astype(DT_NP),
            "idxU1": _idx_tile(iU1_f), "idxU2": _idx_tile(iU2_f),
            "S_U": S_U.astype(DT_NP),
        })

    return nc, in_maps


def kernel(**inputs):
    nc, in_maps = prepare(**inputs)
    r = bass_utils.run_bass_kernel_spmd(
        nc, in_maps, core_ids=list(range(NCORES)))
    node_out = np.concatenate([r.results[c]["node_out"] for c in range(NCORES)], 0)
    edge_out = np.concatenate([r.results[c]["edge_out"] for c in range(NCORES)], 0)
    return node_out, edge_out
